# revision 12
# baseline (speedup 1.0000x reference)
"""Trainium2 Bass kernel for nn_DNCM: out[b] = mean over (HW, C) of ((x @ P) @ Tm[b]) @ Q.

Math: the whole chain after loading I is linear, so
    out[b] = (1/(3*H*W)) * sum_c s[b,c] * w[b,c]
where s[b,c] = sum_{h,w} I[b,c,h,w]  (the memory-bound part: 48MB of reads)
and   w[b]   = P @ Tm[b] @ Q @ ones(3)  (tiny, computed on-chip while streaming).

Sharding: pure data parallel — batch 16 split 2-per-core across 8 cores.

Per-core structure:
  - stream the 6 MB shard as 12 half-plane chunks of (128, 1024) f32,
    DMAs alternating between the two HWDGE rings (SP / ACT);
  - free-dim reduce each chunk to one column of a (128, 12) partials tile,
    alternating between the DVE (reduce_sum) and ACT (activation Copy with
    accum_out) so neither vector engine is the bottleneck;
  - tiny w[b,c] chain runs on partition 0, traced after the loop so the DVE
    queue drains chunk reduces first;
  - tail: PE matmul ones^T @ partials -> (1,12) in PSUM, weight by w, reduce,
    DMA the (2,1) result out.
"""

import sys

for _p in ("/opt/trn_rl_repo", "/root/.axon_site/_ro/trn_rl_repo"):
    if _p not in sys.path:
        sys.path.append(_p)

import numpy as np

import concourse.bass as bass
import concourse.tile as tile
from concourse import bacc
from concourse import mybir

# Full-problem shapes (hardcoded per contract)
BS, C, H, W = 16, 3, 512, 512
K = 16
N_CORES = 8
B_LOC = BS // N_CORES  # 2 batches per core
HW = H * W  # 262144
P_PART = 128
N_PLANES = B_LOC * C  # 6
SPLIT = 2  # chunks per plane
N_CHUNKS = N_PLANES * SPLIT  # 12
FREE = HW // P_PART // SPLIT  # 1024
SCALE = 1.0 / (C * HW)

_CACHED_NC = None


def _build_nc():
    f32 = mybir.dt.float32
    X = mybir.AxisListType.X
    Copy = mybir.ActivationFunctionType.Copy

    nc = bacc.Bacc("TRN2", target_bir_lowering=False, debug=False)

    I_t = nc.dram_tensor("I", [B_LOC, C, H, W], f32, kind="ExternalInput")
    T_t = nc.dram_tensor("T", [B_LOC, K * K], f32, kind="ExternalInput")
    P_t = nc.dram_tensor("P", [C, K], f32, kind="ExternalInput")
    Q_t = nc.dram_tensor("Q", [K, C], f32, kind="ExternalInput")
    out_t = nc.dram_tensor("out", [B_LOC, 1], f32, kind="ExternalOutput")

    with tile.TileContext(nc) as tc:
        with (
            tc.tile_pool(name="const", bufs=1) as const,
            tc.tile_pool(name="stream", bufs=8) as stream,
            tc.tile_pool(name="scratch", bufs=2) as scratch,
            tc.tile_pool(name="psum", bufs=1, space="PSUM") as psum,
        ):
            ones_col = const.tile([P_PART, 1], f32)
            nc.vector.memset(ones_col[:], 1.0)

            # tiny loads on SWDGE so the HWDGE rings only carry the big chunks
            tT = const.tile([1, B_LOC * K * K], f32)  # (1, 512) [b,i,j]
            tP = const.tile([1, C * K], f32)  # (1, 48) [c,i]
            tQ = const.tile([1, K * C], f32)  # (1, 48) [j,c]
            nc.gpsimd.dma_start(tT[:], T_t.ap().rearrange("b k -> (b k)")[None, :])
            nc.gpsimd.dma_start(tP[:], P_t.ap().rearrange("c i -> (c i)")[None, :])
            nc.gpsimd.dma_start(tQ[:], Q_t.ap().rearrange("j c -> (j c)")[None, :])

            # ---------------- streaming reduction over 12 chunks
            # planes 0-4: two (128, 1024) halves; plane 5: (128, 1536) +
            # (128, 512) so the reduce left after the very last DMA is small.
            partials = const.tile([P_PART, N_CHUNKS], f32)
            flat = I_t.ap().rearrange("b c h w -> (b c h w)")
            PLANE_ELEMS = HW
            # (offset_elems, ncols, use_dve) in DMA issue order. Planes 0-3:
            # (1024,1024) halves alternating DVE/ACT; the last two planes are
            # arranged so the big 1536 chunk lands 3rd-from-last and the final
            # DMA is a cheap 512-col chunk — keeps the post-stream reduce tail
            # short on both engines.
            chunk_list = []
            for plane in range(4):
                base = plane * PLANE_ELEMS
                chunk_list.append((base, FREE, True))
                chunk_list.append((base + P_PART * FREE, FREE, False))
            b4, b5 = 4 * PLANE_ELEMS, 5 * PLANE_ELEMS
            chunk_list += [
                (b4, FREE, True),                               # p4a 1024 DVE
                (b5, 3 * FREE // 2, False),                     # p5a 1536 ACT
                (b4 + P_PART * FREE, FREE, True),               # p4b 1024 DVE
                (b5 + P_PART * (3 * FREE // 2), FREE // 2, False),  # p5b 512 ACT
            ]
            for idx, (off, ncols, use_dve) in enumerate(chunk_list):
                xt = stream.tile([P_PART, ncols], f32, tag="xt")
                src = flat[off : off + P_PART * ncols].rearrange(
                    "(p m) -> p m", p=P_PART
                )
                # all chunk DMAs on the SP ring: the ACT sequencer must stay
                # free for the activation-reduces (its DMA dispatches would
                # otherwise queue behind them and land the last chunks late)
                nc.sync.dma_start(xt[:], src)
                if use_dve:
                    nc.vector.reduce_sum(partials[:, idx : idx + 1], xt[:], axis=X)
                else:
                    sc = scratch.tile([P_PART, ncols], f32, tag="sc")
                    nc.scalar.activation(
                        sc[:], xt[:], Copy, accum_out=partials[:, idx : idx + 1]
                    )

            # ---------------- w[b,c] = scale * (P @ Tm[b] @ Q @ 1)[c], partition 0
            # q[j] = sum_c Q[j,c]
            q = const.tile([1, K], f32)
            nc.vector.reduce_sum(q[:], tQ[:].rearrange("p (j c) -> p j c", c=C), axis=X)

            # tqf[b,i,j] = Tm[b,i,j] * q[j]
            tqf = const.tile([1, B_LOC * K * K], f32)
            nc.vector.tensor_mul(
                tqf[:].rearrange("p (b i j) -> p b i j", b=B_LOC, i=K),
                tT[:].rearrange("p (b i j) -> p b i j", b=B_LOC, i=K),
                q[:][:, None, None, :].broadcast_to((1, B_LOC, K, K)),
            )
            # tq[b,i] = sum_j tqf[b,i,j]
            tq = const.tile([1, B_LOC * K], f32)
            nc.vector.reduce_sum(
                tq[:], tqf[:].rearrange("p (bi j) -> p bi j", j=K), axis=X
            )

            # wtmp[b,c,i] = P[c,i] * tq[b,i]
            wtmp = const.tile([1, B_LOC * C * K], f32)
            nc.vector.tensor_mul(
                wtmp[:].rearrange("p (b c i) -> p b c i", b=B_LOC, c=C),
                tP[:].rearrange("p (c i) -> p c i", c=C)[:, None, :, :].broadcast_to(
                    (1, B_LOC, C, K)
                ),
                tq[:].rearrange("p (b i) -> p b i", b=B_LOC)[:, :, None, :].broadcast_to(
                    (1, B_LOC, C, K)
                ),
            )
            # w[b,c] = scale * sum_i wtmp[b,c,i]
            w = const.tile([1, B_LOC * C], f32)
            nc.vector.reduce_sum(
                w[:], wtmp[:].rearrange("p (bc i) -> p bc i", i=K), axis=X
            )
            nc.vector.tensor_scalar_mul(w[:], w[:], SCALE)
            # per-chunk weights following the chunk->plane order
            # [0,0,1,1,2,2,3,3,4,5,4,5]
            w12 = const.tile([1, N_CHUNKS], f32)
            nc.vector.tensor_copy(
                w12[:, 0:8].rearrange("p (bc s) -> p bc s", s=SPLIT),
                w[:, 0:4][:, :, None].broadcast_to((1, 4, SPLIT)),
            )
            nc.vector.tensor_copy(
                w12[:, 8:12].rearrange("p (r pl) -> p r pl", r=2),
                w[:, 4:6][:, None, :].broadcast_to((1, 2, 2)),
            )

            # ---------------- tail: s = ones^T @ partials; out[b] = sum s*w
            psum_s = psum.tile([1, N_CHUNKS], f32)
            nc.tensor.matmul(psum_s[:], ones_col[:], partials[:], start=True, stop=True)

            sw = const.tile([1, N_CHUNKS], f32)
            nc.vector.tensor_mul(sw[:], psum_s[:], w12[:])

            res = const.tile([1, B_LOC], f32)
            nc.vector.reduce_sum(
                res[:], sw[:].rearrange("p (b cs) -> p b cs", b=B_LOC), axis=X
            )
            nc.scalar.dma_start(out_t.ap().rearrange("b o -> (b o)")[None, :], res[:])

    nc.compile()
    return nc


def get_nc():
    global _CACHED_NC
    if _CACHED_NC is None:
        _CACHED_NC = _build_nc()
    return _CACHED_NC


def kernel(I, T, P, Q, **_unused):
    from concourse.bass_utils import run_bass_kernel_spmd

    I = np.ascontiguousarray(I, dtype=np.float32)
    T = np.ascontiguousarray(T, dtype=np.float32)
    P = np.ascontiguousarray(P, dtype=np.float32)
    Q = np.ascontiguousarray(Q, dtype=np.float32)

    nc = get_nc()
    in_maps = [
        {
            "I": I[k * B_LOC : (k + 1) * B_LOC],
            "T": T[k * B_LOC : (k + 1) * B_LOC],
            "P": P,
            "Q": Q,
        }
        for k in range(N_CORES)
    ]
    r = run_bass_kernel_spmd(nc, in_maps, core_ids=list(range(N_CORES)))
    return np.concatenate([r.results[k]["out"] for k in range(N_CORES)], axis=0)
